# revision 1
# baseline (speedup 1.0000x reference)
"""Multi-head attention (B=2, S=2048, E=1024, H=16, Dh=64) on 8 TRN2 NeuronCores.

Sharding: batch x head-group data/tensor parallel. Core c handles batch c//4
and heads [4*(c%4), 4*(c%4)+4): it computes Q/K/V projections for its 256
feature columns, full attention for its 4 heads, and a partial output
projection against its 256 rows of W_o. The host sums the 4 partials per
batch (the "all-reduce after W_o" step of the sharding hint, done at
unshard time) and concatenates the two batches.

Numerics: the softmax here is extremely sharp (logit std ~1000), so the
score path (x -> Q,K -> scores) is computed with an exact fp16 hi/lo
decomposition (x = hi + lo both fp16, dropped lo*lo term ~2^-22) at full
PE rate; the two cross terms are evaluated in ONE K=128 matmul by stacking
[lo;hi] against [hi;lo] along the contraction axis. The row max m is taken
from a hi-only q-major score pass (exact to within a few units, safely
inside the exp window) and subtracted inside the k-major score matmul via
an augmented contraction row (ones x -m), so exp() needs no bias plumbing
and directly fuses the PSUM->SBUF copy on ScalarE. The softmax denominator
comes free from an appended ones-column on V; normalization is applied
after the P@V matmul on the [Dh, S] result. Post-softmax matmuls (V
projection, P@V, W_o) use float32r (~2^-12) / fp16, which is plenty after
the softmax.
"""

from contextlib import ExitStack

import numpy as np

import concourse.bacc as bacc
import concourse.mybir as mybir
import concourse.tile as tile
from concourse import bass_utils
from concourse.masks import make_identity

AF = mybir.ActivationFunctionType
ALU = mybir.AluOpType
F32 = mybir.dt.float32
F16 = mybir.dt.float16
F32R = mybir.dt.float32r

B, S, E, H, Dh = 2, 2048, 1024, 16, 64
NCORES = 8
GROUPS = 4            # head groups (cores per batch)
HPC = H // GROUPS     # heads per core = 4
FG = HPC * Dh         # feature columns per core = 256
P = 128
SCALE = 1.0 / (Dh ** 0.5)

EO = E // P           # 8 contraction chunks
ST = S // P           # 16 sequence tiles of 128
QC = 256              # q-chunk width for the k-major score pass
NQC = S // QC         # 8


def _emit(tc, debug=False):
    nc = tc.nc
    xt_hi = nc.dram_tensor("xt_hi", [E, S], F16, kind="ExternalInput").ap()
    xt_lo = nc.dram_tensor("xt_lo", [E, S], F16, kind="ExternalInput").ap()
    xt_r = nc.dram_tensor("xt_r", [E, S], F32R, kind="ExternalInput").ap()
    wq_hi = nc.dram_tensor("wq_hi", [E, FG], F16, kind="ExternalInput").ap()
    wq_lo = nc.dram_tensor("wq_lo", [E, FG], F16, kind="ExternalInput").ap()
    wk_hi = nc.dram_tensor("wk_hi", [E, FG], F16, kind="ExternalInput").ap()
    wk_lo = nc.dram_tensor("wk_lo", [E, FG], F16, kind="ExternalInput").ap()
    wv = nc.dram_tensor("wv", [E, FG], F32R, kind="ExternalInput").ap()
    wo = nc.dram_tensor("wo", [FG, E], F32R, kind="ExternalInput").ap()
    out = nc.dram_tensor("out", [S, E], F32, kind="ExternalOutput").ap()

    ctx = ExitStack()
    const = ctx.enter_context(tc.tile_pool(name="const", bufs=1))
    persist = ctx.enter_context(tc.tile_pool(name="persist", bufs=1))
    stage = ctx.enter_context(tc.tile_pool(name="stage", bufs=3))
    shp = ctx.enter_context(tc.tile_pool(name="shp", bufs=4))
    ptp = ctx.enter_context(tc.tile_pool(name="ptp", bufs=2))
    outp = ctx.enter_context(tc.tile_pool(name="outp", bufs=4))
    ps_big = ctx.enter_context(tc.tile_pool(name="ps_big", bufs=2, space="PSUM"))
    ps_stat = ctx.enter_context(tc.tile_pool(name="ps_stat", bufs=2, space="PSUM"))
    ps_st = ctx.enter_context(tc.tile_pool(name="ps_st", bufs=2, space="PSUM"))
    ps_pv = ctx.enter_context(tc.tile_pool(name="ps_pv", bufs=2, space="PSUM"))

    ident = const.tile([P, P], F32)
    make_identity(nc, ident[:])
    ones_f32 = const.tile([P, Dh], F32)
    nc.gpsimd.memset(ones_f32[:], 1.0)
    ones_mat = const.tile([P, Dh], F32R)
    nc.vector.tensor_copy(ones_mat[:], ones_f32[:])

    # persistent SBUF tensors
    wqh = persist.tile([P, EO, FG], F16)
    wql = persist.tile([P, EO, FG], F16)
    wkh = persist.tile([P, EO, FG], F16)
    wkl = persist.tile([P, EO, FG], F16)
    wvs = persist.tile([P, EO, FG], F32R)
    wos = persist.tile([P, FG // P, E], F32R)
    # per-head Q^T/K^T hi tiles (partitions 0-63 data, row 64 = -m / ones)
    qhi = persist.tile([P, HPC, S], F16)
    khi = persist.tile([P, HPC, S], F16)
    # cross tiles for the lo*hi + hi*lo term (one K=128 matmul)
    qcr = persist.tile([P, HPC, S], F16)   # 0:64 = Q lo, 64:128 = Q hi
    kcr = persist.tile([P, HPC, S], F16)   # 0:64 = K hi, 64:128 = K lo
    # V with appended ones column (even heads: [V,1], odd heads: [1,V])
    vau = persist.tile([P, ST, HPC, Dh + 1], F16)
    # normalized attention output, feature-major: feature fc*128+p, q free
    att = persist.tile([P, FG // P, S], F32R)

    # ---- load inputs
    xthi_re = xt_hi.rearrange("(eo p) s -> p eo s", p=P)
    xtlo_re = xt_lo.rearrange("(eo p) s -> p eo s", p=P)
    xtr_re = xt_r.rearrange("(eo p) s -> p eo s", p=P)
    _wkh_re = wk_hi.rearrange("(eo p) m -> p eo m", p=P)
    _wkl_re = wk_lo.rearrange("(eo p) m -> p eo m", p=P)
    nc.sync.dma_start(wkh[:, 0, :], _wkh_re[:, 0, :])
    nc.sync.dma_start(wkl[:, 0, :], _wkl_re[:, 0, :])
    nc.sync.dma_start(wkh[:, 1:, :], _wkh_re[:, 1:, :])
    nc.sync.dma_start(wkl[:, 1:, :], _wkl_re[:, 1:, :])


    # K-aug row holds 1/SCALE so the q-side aug row can store -m*SCALE,
    # keeping it inside fp16 range (raw score maxes reach ~66k > fp16 max)
    nc.gpsimd.memset(khi[Dh : Dh + 1, :, :], 1.0 / SCALE)
    nc.gpsimd.memset(vau[:, :, :, Dh : Dh + 1], 1.0)

    # ---- K then Q projections (2-term fp16 hi/lo). K first so head 0's
    # stats matmuls (which read all of khi) can interleave into the Q pass
    # as soon as each q-chunk of qhi lands.
    xck = ctx.enter_context(tc.tile_pool(name="xck", bufs=2))
    xrp = ctx.enter_context(tc.tile_pool(name="xrp", bufs=3))

    negms = [
        stage.tile([P, ST], F32, tag=f"negm{h}", name=f"negm{h}") for h in range(HPC)
    ]

    def stats_steps(h):
        negm = negms[h]
        for qt in range(ST):
            hm = stage.tile([P, 4], F32, tag="hm")
            for quarter in range(4):
                ps = ps_stat.tile([P, 512], F32, tag="stat", name="ps_stat")
                yield lambda ps=ps, qt=qt, h=h, quarter=quarter: nc.tensor.matmul(
                    ps[:],
                    lhsT=qhi[0:Dh, h, qt * P : (qt + 1) * P],
                    rhs=khi[0:Dh, h, quarter * 512 : (quarter + 1) * 512],
                    start=True,
                    stop=True,
                )
                nc.vector.reduce_max(
                    hm[:, quarter : quarter + 1], ps[:], axis=mybir.AxisListType.X
                )
            nc.vector.tensor_reduce(
                negm[:, qt : qt + 1], hm[:, 0:4], axis=mybir.AxisListType.X,
                op=ALU.max, negate=True,
            )

    def drain(it, n=1 << 30):
        k = 0
        if it is not None:
            for step in it:
                step()
                k += 1
                if k >= n:
                    break

    stats_its = [stats_steps(h) for h in range(HPC)]

    def proj_qk(w_h, w_l, hi_dst, cr_dst, is_q):
        for qc4 in range(S // 512):  # 4 chunks of 512 q
            xh_c = xck.tile([P, EO, 512], F16, tag="xh")
            xl_c = xck.tile([P, EO, 512], F16, tag="xl")
            if qc4 == 0 and not is_q:
                for e2 in range(0, EO, 2):
                    nc.sync.dma_start(
                        xh_c[:, e2 : e2 + 2, :], xthi_re[:, e2 : e2 + 2, 0:512]
                    )
                    nc.sync.dma_start(
                        xl_c[:, e2 : e2 + 2, :], xtlo_re[:, e2 : e2 + 2, 0:512]
                    )
            else:
                nc.sync.dma_start(xh_c[:], xthi_re[:, :, qc4 * 512 : (qc4 + 1) * 512])
                nc.sync.dma_start(xl_c[:], xtlo_re[:, :, qc4 * 512 : (qc4 + 1) * 512])
            for mc in range(FG // P):   # 2 chunks of 128 cols (2 heads each)
                ps = ps_big.tile([P, 512], F32, tag="big", name="ps_proj")
                n = 0
                for wt, xt in ((w_h, xh_c), (w_h, xl_c), (w_l, xh_c)):
                    for eo in range(EO):
                        nc.tensor.matmul(
                            ps,
                            lhsT=wt[:, eo, mc * P : (mc + 1) * P],
                            rhs=xt[:, eo, :],
                            start=(n == 0),
                            stop=(n == 23),
                        )
                        n += 1
                sh = shp.tile([P, 512], F16, tag="sh")
                sl = shp.tile([P, 512], F16, tag="sl")
                nc.scalar.copy(sh[:], ps)
                nc.vector.tensor_tensor(sl[:], ps, sh[:], ALU.subtract)
                qs = slice(qc4 * 512, (qc4 + 1) * 512)
                for hh in range(2):
                    h = mc * 2 + hh
                    sp = slice(hh * Dh, (hh + 1) * Dh)
                    nc.gpsimd.dma_start(hi_dst[0:Dh, h, qs], sh[sp, :])
                    if is_q:
                        nc.gpsimd.dma_start(cr_dst[Dh : 2 * Dh, h, qs], sh[sp, :])
                        nc.gpsimd.dma_start(cr_dst[0:Dh, h, qs], sl[sp, :])
                    else:
                        nc.gpsimd.dma_start(cr_dst[0:Dh, h, qs], sh[sp, :])
                        nc.gpsimd.dma_start(cr_dst[Dh : 2 * Dh, h, qs], sl[sp, :])
            if qc4 == 0 and not is_q:
                # late weights ride the HWDGE queue behind the first K chunk
                nc.sync.dma_start(wqh[:], wq_hi.rearrange("(eo p) m -> p eo m", p=P))
                nc.sync.dma_start(wql[:], wq_lo.rearrange("(eo p) m -> p eo m", p=P))
                nc.sync.dma_start(wvs[:], wv.rearrange("(eo p) m -> p eo m", p=P))
                nc.sync.dma_start(wos[:], wo.rearrange("(fo p) e -> p fo e", p=P))

    proj_qk(wkh, wkl, khi, kcr, False)
    proj_qk(wqh, wql, qhi, qcr, True)

    # ---- V projection (f32r), head 1 stats interleaved
    for st in range(ST):
        xr_c = xrp.tile([P, EO, P], F32R, tag="xr")
        nc.sync.dma_start(xr_c[:], xtr_re[:, :, st * P : (st + 1) * P])
        ps = ps_big.tile([P, 512], F32, tag="big", name="ps_v")[:, :FG]
        for eo in range(EO):
            nc.tensor.matmul(
                ps,
                lhsT=xr_c[:, eo, :],
                rhs=wvs[:, eo, :],
                start=(eo == 0),
                stop=(eo == EO - 1),
            )
        drain(stats_its[1], 4)
        nc.scalar.copy(
            vau[:, st, :, 0:Dh],
            ps.rearrange("p (h d) -> p h d", h=HPC),
        )
    drain(stats_its[1])

    # ---- per head: m-transpose, then k-major scores/exp/PV; head h's loop
    # interleaves head h+2's stats; the last head interleaves W_o + output
    def emit_wo(qt):
        for ec in range(E // 512):
            ps = ps_big.tile([P, 512], F32, tag="big", name="ps_wo")
            for fc in range(FG // P):
                nc.tensor.matmul(
                    ps,
                    lhsT=att[:, fc, qt * P : (qt + 1) * P],
                    rhs=wos[:, fc, ec * 512 : (ec + 1) * 512],
                    start=(fc == 0),
                    stop=(fc == FG // P - 1),
                )
            ob = outp.tile([P, 512], F32, tag="ob")
            nc.vector.tensor_copy(ob[:], ps)
            nc.sync.dma_start(
                out[qt * P : (qt + 1) * P, ec * 512 : (ec + 1) * 512], ob[:]
            )

    head_order = [1, 3, 0, 2]
    for hi_idx, h in enumerate(head_order):
        stats_it = stats_its[head_order[hi_idx + 1]] if hi_idx + 1 < HPC else None
        negm = negms[h]
        psm = ps_pv.tile([P, QC], F32, tag="pv", name="psm")
        nc.tensor.transpose(psm[0:ST, 0:P], negm[:, :], ident[:])
        mst = stage.tile([ST, P], F16, tag="mst")
        nc.scalar.mul(mst[:], psm[0:ST, 0:P], SCALE)
        for j in range(ST):
            nc.sync.dma_start(qhi[Dh : Dh + 1, h, j * P : (j + 1) * P], mst[j : j + 1, :])

        for qc in range(NQC):
            qs = slice(qc * QC, (qc + 1) * QC)
            pt = ptp.tile([P, ST * QC], F16)
            pv = ps_pv.tile([P, QC], F32, tag="pv")
            for kc2 in range(ST // 2):
                ps = ps_st.tile([P, 2 * QC], F32, tag="st")
                for sub in range(2):
                    kc = kc2 * 2 + sub
                    ks = slice(kc * P, (kc + 1) * P)
                    pslice = ps[:, sub * QC : (sub + 1) * QC]
                    nc.tensor.matmul(
                        pslice, lhsT=khi[0 : Dh + 1, h, ks], rhs=qhi[0 : Dh + 1, h, qs],
                        start=True, stop=False,
                    )
                    nc.tensor.matmul(
                        pslice, lhsT=kcr[:, h, ks], rhs=qcr[:, h, qs],
                        start=False, stop=True,
                    )
                nc.scalar.activation(
                    pt[:, kc2 * 2 * QC : (kc2 + 1) * 2 * QC], ps[:], AF.Exp, scale=SCALE
                )
                if stats_it is not None:
                    drain(stats_it, 1)
                for sub in range(2):
                    kc = kc2 * 2 + sub
                    nc.tensor.matmul(
                        pv[0 : Dh + 1, :],
                        lhsT=vau[:, kc, h, :],
                        rhs=pt[:, kc * QC : (kc + 1) * QC],
                        start=(kc == 0),
                        stop=(kc == ST - 1),
                        skip_group_check=True,
                    )
            li = stage.tile([P, QC], F32R, tag="li")
            with nc.allow_low_precision(reason="1/l in f32r (~2^-12) is ample"):
                nc.vector.reciprocal(li[Dh : Dh + 1, :], pv[Dh : Dh + 1, :])
            pb = ps_pv.tile([P, QC], F32, tag="pv")
            nc.tensor.matmul(
                pb[0:Dh, :], lhsT=ones_mat[Dh : Dh + 1, :], rhs=li[Dh : Dh + 1, :],
                start=True, stop=True,
            )
            bc = stage.tile([P, QC], F32, tag="bc")
            nc.scalar.copy(bc[0:Dh, :], pb[0:Dh, :])
            if h % 2 == 0:
                nc.vector.tensor_tensor(
                    att[0:Dh, h // 2, qs], pv[0:Dh, :], bc[0:Dh, :], ALU.mult
                )
            else:
                stg = stage.tile([P, QC], F32R, tag="stg")
                nc.vector.tensor_tensor(stg[0:Dh, :], pv[0:Dh, :], bc[0:Dh, :], ALU.mult)
                nc.sync.dma_start(att[Dh : 2 * Dh, h // 2, qs], stg[0:Dh, :])
            if hi_idx == HPC - 1:
                emit_wo(2 * qc)
                emit_wo(2 * qc + 1)
        drain(stats_it)

    if debug:
        att_d = nc.dram_tensor("att_d", [P, FG // P, S], F32, kind="ExternalOutput").ap()
        qhi_d = nc.dram_tensor("qhi_d", [P, HPC, S], F16, kind="ExternalOutput").ap()
        khi_d = nc.dram_tensor("khi_d", [P, HPC, S], F16, kind="ExternalOutput").ap()
        nc.sync.dma_start(att_d, att[:].bitcast(F32))
        nc.sync.dma_start(qhi_d, qhi[:])
        nc.sync.dma_start(khi_d, khi[:])
    ctx.close()


_NC = None


def _build(debug=False):
    global _NC
    if debug:
        nc = bacc.Bacc(
            "TRN2", target_bir_lowering=False, debug=False, num_devices=NCORES
        )
        with tile.TileContext(nc) as tc:
            _emit(tc, debug=True)
        nc.compile()
        return nc
    if _NC is None:
        nc = bacc.Bacc(
            "TRN2", target_bir_lowering=False, debug=False, num_devices=NCORES
        )
        with tile.TileContext(nc) as tc:
            _emit(tc)
        nc.compile()
        _NC = nc
    return _NC


def _prep_inputs(x, W_q, W_k, W_v, W_o):
    x = np.asarray(x, dtype=np.float32)
    W_q = np.asarray(W_q, dtype=np.float32)
    W_k = np.asarray(W_k, dtype=np.float32)
    W_v = np.asarray(W_v, dtype=np.float32)
    W_o = np.asarray(W_o, dtype=np.float32)

    def split16(a):
        hi = a.astype(np.float16)
        lo = (a - hi.astype(np.float32)).astype(np.float16)
        return hi, lo

    per_batch = []
    for b in range(B):
        xt = np.ascontiguousarray(x[b].T)  # [E, S]
        xt_hi, xt_lo = split16(xt)
        per_batch.append((xt_hi, xt_lo, xt))

    in_maps = []
    for c in range(NCORES):
        b, g = divmod(c, GROUPS)
        fg = slice(g * FG, (g + 1) * FG)
        xt_hi, xt_lo, xt = per_batch[b]
        wq_hi, wq_lo = split16(np.ascontiguousarray(W_q[:, fg]))
        wk_hi, wk_lo = split16(np.ascontiguousarray(W_k[:, fg]))
        in_maps.append(
            {
                "xt_hi": xt_hi,
                "xt_lo": xt_lo,
                "xt_r": xt,
                "wq_hi": wq_hi,
                "wq_lo": wq_lo,
                "wk_hi": wk_hi,
                "wk_lo": wk_lo,
                "wv": np.ascontiguousarray(W_v[:, fg]),
                "wo": np.ascontiguousarray(W_o[fg, :]),
            }
        )
    return in_maps


def run(inputs, **spmd_kwargs):
    nc = _build()
    in_maps = _prep_inputs(
        inputs["x"], inputs["W_q"], inputs["W_k"], inputs["W_v"], inputs["W_o"]
    )
    res = bass_utils.run_bass_kernel_spmd(
        nc, in_maps, core_ids=list(range(NCORES)), **spmd_kwargs
    )
    out = np.zeros((B, S, E), dtype=np.float32)
    for c in range(NCORES):
        out[c // GROUPS] += res.results[c]["out"]
    return out, res


def kernel(**inputs):
    out, _ = run(inputs)
    return out



# revision 5
# speedup vs baseline: 1.0594x; 1.0594x over previous
"""Multi-head attention (B=2, S=2048, E=1024, H=16, Dh=64) on 8 TRN2 NeuronCores.

Sharding: batch x head-group data/tensor parallel. Core c handles batch c//4
and heads [4*(c%4), 4*(c%4)+4): it computes Q/K/V projections for its 256
feature columns, full attention for its 4 heads, and a partial output
projection against its 256 rows of W_o. The host sums the 4 partials per
batch (the "all-reduce after W_o" step of the sharding hint, done at
unshard time) and concatenates the two batches.

Numerics: the whole pre-softmax path runs in float32r (~2^-12 per-element
input rounding, fp32 accumulate). The resulting score error is ~0.3 in
scaled-score units; a noise study against the reference shows that level
of score noise costs ~8e-3 output Frobenius error (gate is 2e-2). The row
max m comes from a q-major f32r score pass reduced on DVE with fused
tensor_tensor_reduce pairs; it only needs to land within ~80 raw units of
the true max (any common shift cancels in softmax normalization). The
k-major score matmul subtracts m via an augmented contraction row
(kT row 64 = 1, qT row 64 = -m), so exp() fuses the PSUM->SBUF copy on
ScalarE with scale=1/sqrt(Dh). The softmax denominator comes free from an
appended ones-column on V; normalization is applied after the P@V matmul.
P is fp16 (post-softmax weights), V/att/W_o are f32r.
"""

from contextlib import ExitStack

import numpy as np

import concourse.bacc as bacc
import concourse.mybir as mybir
import concourse.tile as tile
from concourse import bass_utils
from concourse.masks import make_identity

AF = mybir.ActivationFunctionType
ALU = mybir.AluOpType
F32 = mybir.dt.float32
F16 = mybir.dt.float16
F32R = mybir.dt.float32r

B, S, E, H, Dh = 2, 2048, 1024, 16, 64
NCORES = 8
GROUPS = 4            # head groups (cores per batch)
HPC = H // GROUPS     # heads per core = 4
FG = HPC * Dh         # feature columns per core = 256
P = 128
SCALE = 1.0 / (Dh ** 0.5)

EO = E // P           # 8 contraction chunks
ST = S // P           # 16 sequence tiles of 128
QC = 512              # q-chunk width for the k-major score/PV pass
NQC = S // QC         # 4
NEG_INF = -3.0e38


def _emit(tc, debug=False):
    nc = tc.nc
    xt = nc.dram_tensor("xt", [E, S], F32R, kind="ExternalInput").ap()
    wq = nc.dram_tensor("wq", [E, FG], F32R, kind="ExternalInput").ap()
    wk = nc.dram_tensor("wk", [E, FG], F32R, kind="ExternalInput").ap()
    wv = nc.dram_tensor("wv", [E, FG], F32R, kind="ExternalInput").ap()
    wo = nc.dram_tensor("wo", [FG, E], F32R, kind="ExternalInput").ap()
    out = nc.dram_tensor("out", [S, E], F32, kind="ExternalOutput").ap()

    ctx = ExitStack()
    const = ctx.enter_context(tc.tile_pool(name="const", bufs=1))
    persist = ctx.enter_context(tc.tile_pool(name="persist", bufs=1))
    stage = ctx.enter_context(tc.tile_pool(name="stage", bufs=3))
    xqp = ctx.enter_context(tc.tile_pool(name="xqp", bufs=2))
    stgp = ctx.enter_context(tc.tile_pool(name="stgp", bufs=3))
    ptp = ctx.enter_context(tc.tile_pool(name="ptp", bufs=4))
    outp = ctx.enter_context(tc.tile_pool(name="outp", bufs=4))
    ps_stat = ctx.enter_context(tc.tile_pool(name="ps_stat", bufs=2, space="PSUM"))
    ps_sc = ctx.enter_context(tc.tile_pool(name="ps_sc", bufs=2, space="PSUM"))
    ps_pv = ctx.enter_context(tc.tile_pool(name="ps_pv", bufs=2, space="PSUM"))
    ps_mix = ctx.enter_context(tc.tile_pool(name="ps_mix", bufs=2, space="PSUM"))

    ident = const.tile([P, P], F32)
    make_identity(nc, ident[:])
    ones_f32 = const.tile([P, Dh], F32)
    nc.gpsimd.memset(ones_f32[:], 1.0)
    ones_mat = const.tile([P, Dh], F32R)
    nc.vector.tensor_copy(ones_mat[:], ones_f32[:])

    # persistent SBUF tensors
    wqs = persist.tile([P, EO, FG], F32R)
    wks = persist.tile([P, EO, FG], F32R)
    wvs = persist.tile([P, EO, FG], F32R)
    wos = persist.tile([P, FG // P, E], F32R)
    # per-head Q^T/K^T (partitions 0-63 data; row 64 = -m on Q, ones on K)
    qT = persist.tile([P, HPC, S], F32R)
    kT = persist.tile([P, HPC, S], F32R)
    # V with appended ones column, k-major (partitions = sequence position)
    vau = persist.tile([P, ST, HPC, Dh + 1], F16)
    # normalized attention output, feature-major: feature fc*128+p, q free
    att = persist.tile([P, FG // P, S], F32R)

    xt_re = xt.rearrange("(eo p) s -> p eo s", p=P)
    _wks_re = wk.rearrange("(eo p) m -> p eo m", p=P)
    nc.sync.dma_start(wks[:, 0:2, :], _wks_re[:, 0:2, :])
    nc.sync.dma_start(wks[:, 2:, :], _wks_re[:, 2:, :])

    # K-aug row = 1 so the q-side aug row (-m, raw units) lands in every score
    nc.gpsimd.memset(kT[Dh : Dh + 1, :, :].bitcast(F32), 1.0)
    nc.gpsimd.memset(vau[:, :, :, Dh : Dh + 1], 1.0)

    maxs = [
        stage.tile([P, ST], F32, tag=f"maxs{h}", name=f"maxs{h}") for h in range(HPC)
    ]

    # q-major f32r score pass for the row max; DVE fused pair-max reduce
    def stats_steps(h):
        mx = maxs[h]
        for qt in range(ST):
            hm = stage.tile([P, 4], F32, tag="hm")
            for kc in range(4):
                ps = ps_stat.tile([P, 512], F32, tag="stat", name="ps_stat")
                yield lambda ps=ps, qt=qt, h=h, kc=kc: nc.tensor.matmul(
                    ps[:],
                    lhsT=qT[0:Dh, h, qt * P : (qt + 1) * P],
                    rhs=kT[0:Dh, h, kc * 512 : (kc + 1) * 512],
                    start=True,
                    stop=True,
                )
                nc.vector.reduce_max(
                    hm[:, kc : kc + 1], ps[:], axis=mybir.AxisListType.X
                )
            nc.vector.tensor_reduce(
                mx[:, qt : qt + 1], hm[:, 0:4], axis=mybir.AxisListType.X,
                op=ALU.max,
            )

    def drain(it, n=1 << 30):
        k = 0
        if it is not None:
            for step in it:
                step()
                k += 1
                if k >= n:
                    break

    stats_its = [stats_steps(h) for h in range(HPC)]

    # transpose the per-head row-max vector into the qT aug row (negated)
    def aug_prep(h):
        psm = ps_mix.tile([P, QC], F32, tag="mix", name="psm")
        nc.tensor.transpose(psm[0:ST, 0:P], maxs[h][:, :], ident[:])
        mst = stage.tile([ST, P], F32R, tag="mst")
        nc.scalar.mul(mst[:], psm[0:ST, 0:P], -1.0)
        nc.sync.dma_start(qT[Dh : Dh + 1, h, :], mst[:, :])

    # ---- K projection (4 chunks of 512 q)
    for qc4 in range(NQC):
        xq = xqp.tile([P, EO, QC], F32R, tag="xq")
        qs = slice(qc4 * QC, (qc4 + 1) * QC)
        if qc4 == 0:
            for e2 in range(0, EO, 2):
                nc.sync.dma_start(xq[:, e2 : e2 + 2, :], xt_re[:, e2 : e2 + 2, qs])
        else:
            nc.sync.dma_start(xq[:], xt_re[:, :, qs])
        for mc in range(FG // P):
            ps = ps_sc.tile([P, QC], F32, tag="sc", name="ps_kproj")
            for eo in range(EO):
                nc.tensor.matmul(
                    ps,
                    lhsT=wks[:, eo, mc * P : (mc + 1) * P],
                    rhs=xq[:, eo, :],
                    start=(eo == 0),
                    stop=(eo == EO - 1),
                )
            stg = stgp.tile([P, QC], F32R, tag="stg")
            nc.vector.tensor_copy(stg[:], ps)
            for hh in range(2):
                h = mc * 2 + hh
                nc.sync.dma_start(kT[0:Dh, h, qs], stg[hh * Dh : (hh + 1) * Dh, :])
        if qc4 == 0:
            # late weights ride the HWDGE queue behind the first K chunk
            nc.sync.dma_start(wqs[:], wq.rearrange("(eo p) m -> p eo m", p=P))
            nc.sync.dma_start(wvs[:], wv.rearrange("(eo p) m -> p eo m", p=P))
            nc.sync.dma_start(wos[:], wo.rearrange("(fo p) e -> p fo e", p=P))

    # ---- Q + V projections share the x chunk; heads 0/1 stats interleave
    for qc4 in range(NQC):
        xq = xqp.tile([P, EO, QC], F32R, tag="xq")
        qs = slice(qc4 * QC, (qc4 + 1) * QC)
        nc.sync.dma_start(xq[:], xt_re[:, :, qs])
        for mc in range(FG // P):
            ps = ps_sc.tile([P, QC], F32, tag="sc", name="ps_qproj")
            for eo in range(EO):
                nc.tensor.matmul(
                    ps,
                    lhsT=wqs[:, eo, mc * P : (mc + 1) * P],
                    rhs=xq[:, eo, :],
                    start=(eo == 0),
                    stop=(eo == EO - 1),
                )
            stg = stgp.tile([P, QC], F32R, tag="stg")
            nc.vector.tensor_copy(stg[:], ps)
            for hh in range(2):
                h = mc * 2 + hh
                nc.sync.dma_start(qT[0:Dh, h, qs], stg[hh * Dh : (hh + 1) * Dh, :])
        for st4 in range(4):
            st = qc4 * 4 + st4
            ps = ps_pv.tile([P, QC], F32, tag="pv", name="ps_v")[:, :FG]
            for eo in range(EO):
                nc.tensor.matmul(
                    ps,
                    lhsT=xq[:, eo, st4 * P : (st4 + 1) * P],
                    rhs=wvs[:, eo, :],
                    start=(eo == 0),
                    stop=(eo == EO - 1),
                )
            nc.scalar.copy(
                vau[:, st, :, 0:Dh],
                ps.rearrange("p (h d) -> p h d", h=HPC),
            )
        # heads 0 and 1 stats become ready chunk-by-chunk (16 steps each)
        drain(stats_its[0], 16)
        drain(stats_its[1], 16)
    drain(stats_its[0])
    drain(stats_its[1])
    aug_prep(0)
    aug_prep(1)

    # ---- per head: k-major scores -> exp -> PV; heads 2/3 stats interleave
    def emit_wo(qt):
        for ec in range(E // QC):
            ps = ps_mix.tile([P, QC], F32, tag="mix", name="ps_wo")
            for fc in range(FG // P):
                nc.tensor.matmul(
                    ps,
                    lhsT=att[:, fc, qt * P : (qt + 1) * P],
                    rhs=wos[:, fc, ec * QC : (ec + 1) * QC],
                    start=(fc == 0),
                    stop=(fc == FG // P - 1),
                )
            ob = outp.tile([P, QC], F32, tag="ob")
            nc.vector.tensor_copy(ob[:], ps)
            nc.sync.dma_start(
                out[qt * P : (qt + 1) * P, ec * QC : (ec + 1) * QC], ob[:]
            )

    head_order = [0, 1, 2, 3]
    drain_map = {0: 2, 1: 3, 2: None, 3: None}
    for hi_idx, h in enumerate(head_order):
        nxt = drain_map[h]
        stats_it = stats_its[nxt] if nxt is not None else None
        for qc in range(NQC):
            qs = slice(qc * QC, (qc + 1) * QC)
            pv = ps_pv.tile([P, QC], F32, tag="pv")
            for kt in range(ST):
                ks = slice(kt * P, (kt + 1) * P)
                ps = ps_sc.tile([P, QC], F32, tag="sc")
                nc.tensor.matmul(
                    ps, lhsT=kT[0 : Dh + 1, h, ks], rhs=qT[0 : Dh + 1, h, qs],
                    start=True, stop=True,
                )
                pt = ptp.tile([P, QC], F16, tag="pt")
                nc.scalar.activation(pt[:], ps[:], AF.Exp, scale=SCALE)
                if stats_it is not None:
                    drain(stats_it, 1)
                nc.tensor.matmul(
                    pv[0 : Dh + 1, :],
                    lhsT=vau[:, kt, h, :],
                    rhs=pt[:],
                    start=(kt == 0),
                    stop=(kt == ST - 1),
                    skip_group_check=True,
                )
            li = stage.tile([P, QC], F32R, tag="li")
            with nc.allow_low_precision(reason="1/l in f32r (~2^-12) is ample"):
                nc.vector.reciprocal(li[Dh : Dh + 1, :], pv[Dh : Dh + 1, :])
            pb = ps_mix.tile([P, QC], F32, tag="mix", name="pb")
            nc.tensor.matmul(
                pb[0:Dh, :], lhsT=ones_mat[Dh : Dh + 1, :], rhs=li[Dh : Dh + 1, :],
                start=True, stop=True,
            )
            bc = stage.tile([P, QC], F32, tag="bc")
            nc.scalar.copy(bc[0:Dh, :], pb[0:Dh, :])
            if h % 2 == 0:
                nc.vector.tensor_tensor(
                    att[0:Dh, h // 2, qs], pv[0:Dh, :], bc[0:Dh, :], ALU.mult
                )
            else:
                stg = stage.tile([P, QC], F32R, tag="att_stg")
                nc.vector.tensor_tensor(stg[0:Dh, :], pv[0:Dh, :], bc[0:Dh, :], ALU.mult)
                nc.sync.dma_start(att[Dh : 2 * Dh, h // 2, qs], stg[0:Dh, :])
            if hi_idx == HPC - 1:
                for qt in range(4 * qc, 4 * qc + 4):
                    emit_wo(qt)
        drain(stats_it)
        if nxt is not None:
            aug_prep(nxt)

    if debug:
        att_d = nc.dram_tensor("att_d", [P, FG // P, S], F32, kind="ExternalOutput").ap()
        qT_d = nc.dram_tensor("qT_d", [P, HPC, S], F32, kind="ExternalOutput").ap()
        kT_d = nc.dram_tensor("kT_d", [P, HPC, S], F32, kind="ExternalOutput").ap()
        nc.sync.dma_start(att_d, att[:].bitcast(F32))
        nc.sync.dma_start(qT_d, qT[:].bitcast(F32))
        nc.sync.dma_start(kT_d, kT[:].bitcast(F32))
    ctx.close()


_NC = None


def _build(debug=False):
    global _NC
    if debug:
        nc = bacc.Bacc(
            "TRN2", target_bir_lowering=False, debug=False, num_devices=NCORES
        )
        with tile.TileContext(nc) as tc:
            _emit(tc, debug=True)
        nc.compile()
        return nc
    if _NC is None:
        nc = bacc.Bacc(
            "TRN2", target_bir_lowering=False, debug=False, num_devices=NCORES
        )
        with tile.TileContext(nc) as tc:
            _emit(tc)
        nc.compile()
        _NC = nc
    return _NC


def _prep_inputs(x, W_q, W_k, W_v, W_o):
    x = np.asarray(x, dtype=np.float32)
    W_q = np.asarray(W_q, dtype=np.float32)
    W_k = np.asarray(W_k, dtype=np.float32)
    W_v = np.asarray(W_v, dtype=np.float32)
    W_o = np.asarray(W_o, dtype=np.float32)

    xts = [np.ascontiguousarray(x[b].T) for b in range(B)]
    in_maps = []
    for c in range(NCORES):
        b, g = divmod(c, GROUPS)
        fg = slice(g * FG, (g + 1) * FG)
        in_maps.append(
            {
                "xt": xts[b],
                "wq": np.ascontiguousarray(W_q[:, fg]),
                "wk": np.ascontiguousarray(W_k[:, fg]),
                "wv": np.ascontiguousarray(W_v[:, fg]),
                "wo": np.ascontiguousarray(W_o[fg, :]),
            }
        )
    return in_maps


def run(inputs, **spmd_kwargs):
    nc = _build()
    in_maps = _prep_inputs(
        inputs["x"], inputs["W_q"], inputs["W_k"], inputs["W_v"], inputs["W_o"]
    )
    res = bass_utils.run_bass_kernel_spmd(
        nc, in_maps, core_ids=list(range(NCORES)), **spmd_kwargs
    )
    out = np.zeros((B, S, E), dtype=np.float32)
    for c in range(NCORES):
        out[c // GROUPS] += res.results[c]["out"]
    return out, res


def kernel(**inputs):
    out, _ = run(inputs)
    return out


# revision 14
# speedup vs baseline: 1.0720x; 1.0119x over previous
"""Multi-head attention (B=2, S=2048, E=1024, H=16, Dh=64) on 8 TRN2 NeuronCores.

Sharding: batch x head-group data/tensor parallel. Core c handles batch c//4
and heads [4*(c%4), 4*(c%4)+4): it computes Q/K/V projections for its 256
feature columns, full attention for its 4 heads, and a partial output
projection against its 256 rows of W_o. The host sums the 4 partials per
batch (the "all-reduce after W_o" step of the sharding hint, done at
unshard time) and concatenates the two batches.

Numerics: the whole pre-softmax path runs in float32r (~2^-12 per-element
input rounding, fp32 accumulate). The resulting score error is ~0.3 in
scaled-score units; a noise study against the reference shows that level
of score noise costs ~8e-3 output Frobenius error (gate is 2e-2). The row
max m comes from a q-major f32r score pass reduced on DVE with fused
tensor_tensor_reduce pairs; it only needs to land within ~80 raw units of
the true max (any common shift cancels in softmax normalization). The
k-major score matmul subtracts m via an augmented contraction row
(kT row 64 = 1, qT row 64 = -m), so exp() fuses the PSUM->SBUF copy on
ScalarE with scale=1/sqrt(Dh). The softmax denominator comes free from an
appended ones-column on V; normalization is applied after the P@V matmul.
P is fp16 (post-softmax weights), V/att/W_o are f32r.
"""

from contextlib import ExitStack

import numpy as np

import concourse.bacc as bacc
import concourse.mybir as mybir
import concourse.tile as tile
from concourse import bass_utils
from concourse.masks import make_identity

AF = mybir.ActivationFunctionType
ALU = mybir.AluOpType
F32 = mybir.dt.float32
F16 = mybir.dt.float16
F32R = mybir.dt.float32r

B, S, E, H, Dh = 2, 2048, 1024, 16, 64
NCORES = 8
GROUPS = 4            # head groups (cores per batch)
HPC = H // GROUPS     # heads per core = 4
FG = HPC * Dh         # feature columns per core = 256
P = 128
SCALE = 1.0 / (Dh ** 0.5)

EO = E // P           # 8 contraction chunks
ST = S // P           # 16 sequence tiles of 128
QC = 512              # q-chunk width for the k-major score/PV pass
NQC = S // QC         # 4
NEG_INF = -3.0e38


def _emit(tc, debug=False):
    nc = tc.nc
    xt = nc.dram_tensor("xt", [E, S], F32R, kind="ExternalInput").ap()
    wq = nc.dram_tensor("wq", [E, FG], F32R, kind="ExternalInput").ap()
    wk = nc.dram_tensor("wk", [E, FG], F32R, kind="ExternalInput").ap()
    wv = nc.dram_tensor("wv", [E, FG], F32R, kind="ExternalInput").ap()
    wo = nc.dram_tensor("wo", [FG, E], F32R, kind="ExternalInput").ap()
    out = nc.dram_tensor("out", [S, E], F32, kind="ExternalOutput").ap()

    ctx = ExitStack()
    const = ctx.enter_context(tc.tile_pool(name="const", bufs=1))
    persist = ctx.enter_context(tc.tile_pool(name="persist", bufs=1))
    stage = ctx.enter_context(tc.tile_pool(name="stage", bufs=3))
    xqp = ctx.enter_context(tc.tile_pool(name="xqp", bufs=2))
    stgp = ctx.enter_context(tc.tile_pool(name="stgp", bufs=3))
    ptp = ctx.enter_context(tc.tile_pool(name="ptp", bufs=4))
    outp = ctx.enter_context(tc.tile_pool(name="outp", bufs=4))
    ps_stat = ctx.enter_context(tc.tile_pool(name="ps_stat", bufs=2, space="PSUM"))
    ps_sc = ctx.enter_context(tc.tile_pool(name="ps_sc", bufs=2, space="PSUM"))
    ps_pv = ctx.enter_context(tc.tile_pool(name="ps_pv", bufs=2, space="PSUM"))
    ps_mix = ctx.enter_context(tc.tile_pool(name="ps_mix", bufs=2, space="PSUM"))

    ident = const.tile([P, P], F32)
    make_identity(nc, ident[:])
    ones_f32 = const.tile([P, Dh], F32)
    nc.gpsimd.memset(ones_f32[:], 1.0)
    ones_mat = const.tile([P, Dh], F32R)
    nc.vector.tensor_copy(ones_mat[:], ones_f32[:])

    # persistent SBUF tensors
    wqs = persist.tile([P, EO, FG], F32R)
    wks = persist.tile([P, EO, FG], F32R)
    wvs = persist.tile([P, EO, FG], F32R)
    wos = persist.tile([P, FG // P, E], F32R)
    # per-head Q^T/K^T (partitions 0-63 data; row 64 = -m on Q, ones on K)
    qT = persist.tile([P, HPC, S], F32R)
    kT = persist.tile([P, HPC, S], F32R)
    # V with appended ones column, k-major (partitions = sequence position)
    vau = persist.tile([P, ST, HPC, Dh + 1], F16)
    # normalized attention output, feature-major: feature fc*128+p, q free
    att = persist.tile([P, FG // P, S], F32R)

    xt_re = xt.rearrange("(eo p) s -> p eo s", p=P)
    _wks_re = wk.rearrange("(eo p) m -> p eo m", p=P)
    nc.sync.dma_start(wks[:, 0:2, :], _wks_re[:, 0:2, :])
    nc.sync.dma_start(wks[:, 2:, :], _wks_re[:, 2:, :])

    # K-aug row = 1 so the q-side aug row (-m, raw units) lands in every score
    nc.gpsimd.memset(kT[Dh : Dh + 1, :, :].bitcast(F32), 1.0)
    nc.gpsimd.memset(vau[:, :, :, Dh : Dh + 1], 1.0)

    maxs = [
        stage.tile([P, ST], F32, tag=f"maxs{h}", name=f"maxs{h}") for h in range(HPC)
    ]

    # q-major f32r score pass for the row max; DVE fused pair-max reduce
    def stats_steps(h):
        mx = maxs[h]
        for qt in range(ST):
            hm = stage.tile([P, 4], F32, tag="hm")
            for kc in range(4):
                ps = ps_stat.tile([P, 512], F32, tag="stat", name="ps_stat")
                yield lambda ps=ps, qt=qt, h=h, kc=kc: nc.tensor.matmul(
                    ps[:],
                    lhsT=qT[0:Dh, h, qt * P : (qt + 1) * P],
                    rhs=kT[0:Dh, h, kc * 512 : (kc + 1) * 512],
                    start=True,
                    stop=True,
                )
                nc.vector.reduce_max(
                    hm[:, kc : kc + 1], ps[:], axis=mybir.AxisListType.X
                )
            nc.vector.tensor_reduce(
                mx[:, qt : qt + 1], hm[:, 0:4], axis=mybir.AxisListType.X,
                op=ALU.max,
            )

    def drain(it, n=1 << 30):
        k = 0
        if it is not None:
            for step in it:
                step()
                k += 1
                if k >= n:
                    break

    stats_its = [stats_steps(h) for h in range(HPC)]

    # transpose the per-head row-max vector into the qT aug row (negated)
    def aug_prep(h):
        psm = ps_mix.tile([P, QC], F32, tag="mix", name="psm")
        nc.tensor.transpose(psm[0:ST, 0:P], maxs[h][:, :], ident[:])
        mst = stage.tile([ST, P], F32R, tag="mst")
        nc.scalar.mul(mst[:], psm[0:ST, 0:P], -1.0)
        nc.sync.dma_start(qT[Dh : Dh + 1, h, :], mst[:, :])

    # ---- K projection (4 chunks of 512 q)
    for qc4 in range(NQC):
        xq = xqp.tile([P, EO, QC], F32R, tag="xq")
        qs = slice(qc4 * QC, (qc4 + 1) * QC)
        if qc4 == 0:
            for e2 in range(0, EO, 2):
                nc.sync.dma_start(xq[:, e2 : e2 + 2, :], xt_re[:, e2 : e2 + 2, qs])
        else:
            nc.sync.dma_start(xq[:], xt_re[:, :, qs])
        for mc in range(FG // P):
            ps = ps_sc.tile([P, QC], F32, tag="sc", name="ps_kproj")
            for eo in range(EO):
                nc.tensor.matmul(
                    ps,
                    lhsT=wks[:, eo, mc * P : (mc + 1) * P],
                    rhs=xq[:, eo, :],
                    start=(eo == 0),
                    stop=(eo == EO - 1),
                )
            stg = stgp.tile([P, QC], F32R, tag="stg")
            nc.vector.tensor_copy(stg[:], ps)
            for hh in range(2):
                h = mc * 2 + hh
                nc.sync.dma_start(kT[0:Dh, h, qs], stg[hh * Dh : (hh + 1) * Dh, :])
        if qc4 == 0:
            # late weights ride the HWDGE queue behind the first K chunk
            nc.sync.dma_start(wqs[:], wq.rearrange("(eo p) m -> p eo m", p=P))
            nc.sync.dma_start(wvs[:], wv.rearrange("(eo p) m -> p eo m", p=P))
            nc.sync.dma_start(wos[:], wo.rearrange("(fo p) e -> p fo e", p=P))

    # ---- Q + V projections share the x chunk; heads 0/1 stats interleave
    for qc4 in range(NQC):
        xq = xqp.tile([P, EO, QC], F32R, tag="xq")
        qs = slice(qc4 * QC, (qc4 + 1) * QC)
        nc.sync.dma_start(xq[:], xt_re[:, :, qs])
        for mc in range(FG // P):
            ps = ps_sc.tile([P, QC], F32, tag="sc", name="ps_qproj")
            for eo in range(EO):
                nc.tensor.matmul(
                    ps,
                    lhsT=wqs[:, eo, mc * P : (mc + 1) * P],
                    rhs=xq[:, eo, :],
                    start=(eo == 0),
                    stop=(eo == EO - 1),
                )
            stg = stgp.tile([P, QC], F32R, tag="stg")
            nc.vector.tensor_copy(stg[:], ps)
            for hh in range(2):
                h = mc * 2 + hh
                nc.sync.dma_start(qT[0:Dh, h, qs], stg[hh * Dh : (hh + 1) * Dh, :])
        for st4 in range(4):
            st = qc4 * 4 + st4
            ps = ps_pv.tile([P, QC], F32, tag="pv", name="ps_v")[:, :FG]
            for eo in range(EO):
                nc.tensor.matmul(
                    ps,
                    lhsT=xq[:, eo, st4 * P : (st4 + 1) * P],
                    rhs=wvs[:, eo, :],
                    start=(eo == 0),
                    stop=(eo == EO - 1),
                )
            nc.scalar.copy(
                vau[:, st, :, 0:Dh],
                ps.rearrange("p (h d) -> p h d", h=HPC),
            )
            # heads 0/1 stats become ready chunk-by-chunk; spread the DVE
            # reduces across the V sub-tiles so neither engine bursts
            drain(stats_its[0], 4)
            drain(stats_its[1], 4)
    drain(stats_its[0])
    drain(stats_its[1])
    aug_prep(0)
    aug_prep(1)

    # ---- per head: k-major scores -> exp -> PV, software-pipelined so PE
    # never sits behind the Act exp; heads 2/3 stats and the W_o projection
    # interleave into the PE slack of the Act-bound chain.
    def wo_steps(qt_lo, qt_hi):
        for qt in range(qt_lo, qt_hi):
            for ec in range(E // QC):
                ps = ps_mix.tile([P, QC], F32, tag="mix", name="ps_wo")
                for fc in range(FG // P):
                    yield lambda ps=ps, qt=qt, ec=ec, fc=fc: nc.tensor.matmul(
                        ps,
                        lhsT=att[:, fc, qt * P : (qt + 1) * P],
                        rhs=wos[:, fc, ec * QC : (ec + 1) * QC],
                        start=(fc == 0),
                        stop=(fc == FG // P - 1),
                    )
                ob = outp.tile([P, QC], F32, tag="ob")
                nc.vector.tensor_copy(ob[:], ps)
                nc.sync.dma_start(
                    out[qt * P : (qt + 1) * P, ec * QC : (ec + 1) * QC], ob[:]
                )

    LAG = 2
    head_order = [0, 1, 2, 3]
    # Fill schedule: stats2 must finish by end of head 1 (aug for head 2),
    # stats3 by end of head 2. Spread so DVE's max-reduces never exceed the
    # ~640ns/slot budget of the Act-bound exp chain.
    fill_even = {0: stats_its[2], 1: stats_its[2], 2: stats_its[3]}
    fill_odd = {0: stats_its[3], 1: stats_its[3], 2: stats_its[3]}
    wo_pending = None
    for hi_idx, h in enumerate(head_order):
        for qc in range(NQC):
            qs = slice(qc * QC, (qc + 1) * QC)
            pv = ps_pv.tile([P, QC], F32, tag="pv")
            pts = {}

            def pv_mm(kt, pv=pv, h=h):
                nc.tensor.matmul(
                    pv[0 : Dh + 1, :],
                    lhsT=vau[:, kt, h, :],
                    rhs=pts.pop(kt)[:],
                    start=(kt == 0),
                    stop=(kt == ST - 1),
                    skip_group_check=True,
                )

            for kt in range(ST):
                ks = slice(kt * P, (kt + 1) * P)
                if hi_idx == HPC - 1:
                    drain(wo_pending, 1)
                elif kt % 2 == 0:
                    drain(fill_even[h], 1)
                elif kt % 3 == 1:
                    drain(fill_odd[h], 1)
                ps = ps_sc.tile([P, QC], F32, tag="sc")
                nc.tensor.matmul(
                    ps, lhsT=kT[0 : Dh + 1, h, ks], rhs=qT[0 : Dh + 1, h, qs],
                    start=True, stop=True,
                )
                pt = ptp.tile([P, QC], F16, tag="pt")
                pts[kt] = pt
                nc.scalar.activation(pt[:], ps[:], AF.Exp, scale=SCALE)
                if kt >= LAG:
                    pv_mm(kt - LAG)
            for kt in range(ST - LAG, ST):
                pv_mm(kt)
            li = stage.tile([P, QC], F32R, tag="li")
            with nc.allow_low_precision(reason="1/l in f32r (~2^-12) is ample"):
                nc.vector.reciprocal(li[Dh : Dh + 1, :], pv[Dh : Dh + 1, :])
            pb = ps_mix.tile([P, QC], F32, tag="mix", name="pb")
            nc.tensor.matmul(
                pb[0:Dh, :], lhsT=ones_mat[Dh : Dh + 1, :], rhs=li[Dh : Dh + 1, :],
                start=True, stop=True,
            )
            bc = stage.tile([P, QC], F32, tag="bc")
            nc.scalar.copy(bc[0:Dh, :], pb[0:Dh, :])
            if h % 2 == 0:
                nc.vector.tensor_tensor(
                    att[0:Dh, h // 2, qs], pv[0:Dh, :], bc[0:Dh, :], ALU.mult
                )
            else:
                stg = stage.tile([P, QC], F32R, tag="att_stg")
                nc.vector.tensor_tensor(stg[0:Dh, :], pv[0:Dh, :], bc[0:Dh, :], ALU.mult)
                nc.sync.dma_start(att[Dh : 2 * Dh, h // 2, qs], stg[0:Dh, :])
            if hi_idx == HPC - 1:
                drain(wo_pending)
                wo_pending = wo_steps(4 * qc, 4 * qc + 4)
        if hi_idx == HPC - 1:
            drain(wo_pending)
        if hi_idx + 1 < HPC:
            nxt = head_order[hi_idx + 1]
            if nxt >= 2:
                drain(stats_its[nxt])
                aug_prep(nxt)

    if debug:
        att_d = nc.dram_tensor("att_d", [P, FG // P, S], F32, kind="ExternalOutput").ap()
        qT_d = nc.dram_tensor("qT_d", [P, HPC, S], F32, kind="ExternalOutput").ap()
        kT_d = nc.dram_tensor("kT_d", [P, HPC, S], F32, kind="ExternalOutput").ap()
        nc.sync.dma_start(att_d, att[:].bitcast(F32))
        nc.sync.dma_start(qT_d, qT[:].bitcast(F32))
        nc.sync.dma_start(kT_d, kT[:].bitcast(F32))
    ctx.close()


_NC = None


def _build(debug=False):
    global _NC
    if debug:
        nc = bacc.Bacc(
            "TRN2", target_bir_lowering=False, debug=False, num_devices=NCORES
        )
        with tile.TileContext(nc) as tc:
            _emit(tc, debug=True)
        nc.compile()
        return nc
    if _NC is None:
        nc = bacc.Bacc(
            "TRN2", target_bir_lowering=False, debug=False, num_devices=NCORES
        )
        with tile.TileContext(nc) as tc:
            _emit(tc)
        nc.compile()
        _NC = nc
    return _NC


def _prep_inputs(x, W_q, W_k, W_v, W_o):
    x = np.asarray(x, dtype=np.float32)
    W_q = np.asarray(W_q, dtype=np.float32)
    W_k = np.asarray(W_k, dtype=np.float32)
    W_v = np.asarray(W_v, dtype=np.float32)
    W_o = np.asarray(W_o, dtype=np.float32)

    xts = [np.ascontiguousarray(x[b].T) for b in range(B)]
    in_maps = []
    for c in range(NCORES):
        b, g = divmod(c, GROUPS)
        fg = slice(g * FG, (g + 1) * FG)
        in_maps.append(
            {
                "xt": xts[b],
                "wq": np.ascontiguousarray(W_q[:, fg]),
                "wk": np.ascontiguousarray(W_k[:, fg]),
                "wv": np.ascontiguousarray(W_v[:, fg]),
                "wo": np.ascontiguousarray(W_o[fg, :]),
            }
        )
    return in_maps


def run(inputs, **spmd_kwargs):
    nc = _build()
    in_maps = _prep_inputs(
        inputs["x"], inputs["W_q"], inputs["W_k"], inputs["W_v"], inputs["W_o"]
    )
    res = bass_utils.run_bass_kernel_spmd(
        nc, in_maps, core_ids=list(range(NCORES)), **spmd_kwargs
    )
    out = np.zeros((B, S, E), dtype=np.float32)
    for c in range(NCORES):
        out[c // GROUPS] += res.results[c]["out"]
    return out, res


def kernel(**inputs):
    out, _ = run(inputs)
    return out


# revision 20
# speedup vs baseline: 1.1069x; 1.0326x over previous
"""Multi-head attention (B=2, S=2048, E=1024, H=16, Dh=64) on 8 TRN2 NeuronCores.

Sharding: batch x head-group data/tensor parallel. Core c handles batch c//4
and heads [4*(c%4), 4*(c%4)+4): it computes Q/K/V projections for its 256
feature columns, full attention for its 4 heads, and a partial output
projection against its 256 rows of W_o. The host sums the 4 partials per
batch (the "all-reduce after W_o" step of the sharding hint, done at
unshard time) and concatenates the two batches.

Numerics: the whole pre-softmax path runs in float32r (~2^-12 per-element
input rounding, fp32 accumulate). The resulting score error is ~0.3 in
scaled-score units; a noise study against the reference shows that level
of score noise costs ~8e-3 output Frobenius error (gate is 2e-2). The row
max m comes from a q-major f32r score pass reduced on DVE with fused
tensor_tensor_reduce pairs; it only needs to land within ~80 raw units of
the true max (any common shift cancels in softmax normalization). The
k-major score matmul subtracts m via an augmented contraction row
(kT row 64 = 1, qT row 64 = -m), so exp() fuses the PSUM->SBUF copy on
ScalarE with scale=1/sqrt(Dh). The softmax denominator comes free from an
appended ones-column on V; normalization is applied after the P@V matmul.
P is fp16 (post-softmax weights), V/att/W_o are f32r.
"""

from contextlib import ExitStack

import numpy as np

import concourse.bacc as bacc
import concourse.mybir as mybir
import concourse.tile as tile
from concourse import bass_utils
from concourse.masks import make_identity

AF = mybir.ActivationFunctionType
ALU = mybir.AluOpType
F32 = mybir.dt.float32
F16 = mybir.dt.float16
F32R = mybir.dt.float32r

B, S, E, H, Dh = 2, 2048, 1024, 16, 64
NCORES = 8
GROUPS = 4            # head groups (cores per batch)
HPC = H // GROUPS     # heads per core = 4
FG = HPC * Dh         # feature columns per core = 256
P = 128
SCALE = 1.0 / (Dh ** 0.5)

EO = E // P           # 8 contraction chunks
ST = S // P           # 16 sequence tiles of 128
QC = 512              # q-chunk width for the k-major score/PV pass
NQC = S // QC         # 4
NEG_INF = -3.0e38


def _emit(tc, debug=False):
    nc = tc.nc
    xt = nc.dram_tensor("xt", [E, S], F32R, kind="ExternalInput").ap()
    wq = nc.dram_tensor("wq", [E, FG], F32R, kind="ExternalInput").ap()
    wk = nc.dram_tensor("wk", [E, FG], F32R, kind="ExternalInput").ap()
    wv = nc.dram_tensor("wv", [E, FG], F32R, kind="ExternalInput").ap()
    wo = nc.dram_tensor("wo", [FG, E], F32R, kind="ExternalInput").ap()
    out = nc.dram_tensor("out", [S, E], F32, kind="ExternalOutput").ap()

    ctx = ExitStack()
    const = ctx.enter_context(tc.tile_pool(name="const", bufs=1))
    persist = ctx.enter_context(tc.tile_pool(name="persist", bufs=1))
    stage = ctx.enter_context(tc.tile_pool(name="stage", bufs=3))
    xqp = ctx.enter_context(tc.tile_pool(name="xqp", bufs=2))
    stgp = ctx.enter_context(tc.tile_pool(name="stgp", bufs=3))
    ptp = ctx.enter_context(tc.tile_pool(name="ptp", bufs=4))
    outp = ctx.enter_context(tc.tile_pool(name="outp", bufs=4))
    ps_stat = ctx.enter_context(tc.tile_pool(name="ps_stat", bufs=2, space="PSUM"))
    ps_sc = ctx.enter_context(tc.tile_pool(name="ps_sc", bufs=2, space="PSUM"))
    ps_pv = ctx.enter_context(tc.tile_pool(name="ps_pv", bufs=2, space="PSUM"))

    ident = const.tile([P, P], F32)
    make_identity(nc, ident[:])
    ones_f32 = const.tile([P, Dh], F32)
    nc.gpsimd.memset(ones_f32[:], 1.0)
    ones_mat = const.tile([P, Dh], F32R)
    nc.vector.tensor_copy(ones_mat[:], ones_f32[:])

    # persistent SBUF tensors
    wqs = persist.tile([P, EO, FG], F32R)
    wks = persist.tile([P, EO, FG], F32R)
    wvs = persist.tile([P, EO, FG], F32R)
    wos = persist.tile([P, FG // P, E], F32R)
    # per-head Q^T/K^T (partitions 0-63 data; row 64 = -m on Q, ones on K)
    qT = persist.tile([P, HPC, S], F32R)
    kT = persist.tile([P, HPC, S], F32R)
    # V with appended ones column, k-major (partitions = sequence position)
    vau = persist.tile([P, ST, HPC, Dh + 1], F16)
    # normalized attention output, feature-major: feature fc*128+p, q free
    att = persist.tile([P, FG // P, S], F32R)

    xt_re = xt.rearrange("(eo p) s -> p eo s", p=P)
    _wks_re = wk.rearrange("(eo p) m -> p eo m", p=P)
    nc.sync.dma_start(wks[:, 0:2, :], _wks_re[:, 0:2, :])
    nc.sync.dma_start(wks[:, 2:, :], _wks_re[:, 2:, :])

    # K-aug row = 1 so the q-side aug row (-m, raw units) lands in every score
    nc.gpsimd.memset(kT[Dh : Dh + 1, :, :].bitcast(F32), 1.0)
    nc.gpsimd.memset(vau[:, :, :, Dh : Dh + 1], 1.0)

    maxs = [
        stage.tile([P, ST], F32, tag=f"maxs{h}", name=f"maxs{h}") for h in range(HPC)
    ]

    # q-major f32r score pass for the row max, reduced per 512-block on DVE
    def stats_steps(h, qt_order=None):
        mx = maxs[h]
        for qt in qt_order or range(ST):
            hm = stage.tile([P, 4], F32, tag="hm")
            for kc in range(4):
                ps = ps_stat.tile([P, 512], F32, tag="stat", name="ps_stat")
                yield lambda ps=ps, qt=qt, h=h, kc=kc: nc.tensor.matmul(
                    ps[:],
                    lhsT=qT[0:Dh, h, qt * P : (qt + 1) * P],
                    rhs=kT[0:Dh, h, kc * 512 : (kc + 1) * 512],
                    start=True,
                    stop=True,
                )
                nc.vector.reduce_max(
                    hm[:, kc : kc + 1], ps[:], axis=mybir.AxisListType.X
                )
            nc.vector.tensor_reduce(
                mx[:, qt : qt + 1], hm[:, 0:4], axis=mybir.AxisListType.X,
                op=ALU.max,
            )

    def drain(it, n=1 << 30):
        k = 0
        if it is not None:
            for step in it:
                step()
                k += 1
                if k >= n:
                    break

    Q_CHUNK_ORDER = [2, 3, 1, 0]
    qt0_order = [qc4 * 4 + j for qc4 in Q_CHUNK_ORDER for j in range(4)]
    stats_its = [stats_steps(0, qt0_order)] + [stats_steps(h) for h in range(1, HPC)]

    # transpose the per-head row-max vector into the qT aug row (negated)
    def aug_prep(h):
        psm = ps_sc.tile([P, QC], F32, tag="sc", name="psm")
        nc.tensor.transpose(psm[0:ST, 0:P], maxs[h][:, :], ident[:])
        mst = stage.tile([ST, P], F32R, tag="mst")
        nc.scalar.mul(mst[:], psm[0:ST, 0:P], -1.0)
        nc.sync.dma_start(qT[Dh : Dh + 1, h, :], mst[:, :])

    # ---- phase 1: K + V projections share each x chunk
    xq_tiles = {}
    for qc4 in range(NQC):
        xq = xqp.tile([P, EO, QC], F32R, tag="xq")
        xq_tiles[qc4] = xq
        qs = slice(qc4 * QC, (qc4 + 1) * QC)
        if qc4 == 0:
            for e2 in range(0, EO, 2):
                nc.sync.dma_start(xq[:, e2 : e2 + 2, :], xt_re[:, e2 : e2 + 2, qs])
            # late weights ride the HWDGE queue behind the first x chunk;
            # they must be EMITTED before the first V/Q matmuls that read them
            nc.sync.dma_start(wvs[:], wv.rearrange("(eo p) m -> p eo m", p=P))
            nc.sync.dma_start(wqs[:], wq.rearrange("(eo p) m -> p eo m", p=P))
            nc.sync.dma_start(wos[:], wo.rearrange("(fo p) e -> p fo e", p=P))
        else:
            nc.sync.dma_start(xq[:], xt_re[:, :, qs])
        for mc in range(FG // P):
            ps = ps_sc.tile([P, QC], F32, tag="sc", name="ps_kproj")
            for eo in range(EO):
                nc.tensor.matmul(
                    ps,
                    lhsT=wks[:, eo, mc * P : (mc + 1) * P],
                    rhs=xq[:, eo, :],
                    start=(eo == 0),
                    stop=(eo == EO - 1),
                )
            stg = stgp.tile([P, QC], F32R, tag="stg")
            nc.vector.tensor_copy(stg[:], ps)
            for hh in range(2):
                h = mc * 2 + hh
                nc.sync.dma_start(kT[0:Dh, h, qs], stg[hh * Dh : (hh + 1) * Dh, :])
        for st4 in range(4):
            st = qc4 * 4 + st4
            ps = ps_pv.tile([P, QC], F32, tag="pv", name="ps_v")[:, :FG]
            for eo in range(EO):
                nc.tensor.matmul(
                    ps,
                    lhsT=xq[:, eo, st4 * P : (st4 + 1) * P],
                    rhs=wvs[:, eo, :],
                    start=(eo == 0),
                    stop=(eo == EO - 1),
                )
            nc.scalar.copy(
                vau[:, st, :, 0:Dh],
                ps.rearrange("p (h d) -> p h d", h=HPC),
            )

    # ---- phase 2: Q projection; chunks 2,3 still resident in the xq ring.
    # head-0 stats drain here (16 per chunk, matching qt availability).
    for ci, qc4 in enumerate(Q_CHUNK_ORDER):
        if qc4 in (2, 3):
            xq = xq_tiles[qc4]
        else:
            xq = xqp.tile([P, EO, QC], F32R, tag="xq")
            nc.sync.dma_start(xq[:], xt_re[:, :, qc4 * QC : (qc4 + 1) * QC])
        qs = slice(qc4 * QC, (qc4 + 1) * QC)
        for mc in range(FG // P):
            ps = ps_sc.tile([P, QC], F32, tag="sc", name="ps_qproj")
            for eo in range(EO):
                nc.tensor.matmul(
                    ps,
                    lhsT=wqs[:, eo, mc * P : (mc + 1) * P],
                    rhs=xq[:, eo, :],
                    start=(eo == 0),
                    stop=(eo == EO - 1),
                )
            stg = stgp.tile([P, QC], F32R, tag="stg")
            nc.vector.tensor_copy(stg[:], ps)
            for hh in range(2):
                h = mc * 2 + hh
                nc.sync.dma_start(qT[0:Dh, h, qs], stg[hh * Dh : (hh + 1) * Dh, :])
            drain(stats_its[0], 8)
    drain(stats_its[0])
    aug_prep(0)

    # ---- per head: k-major scores -> exp -> PV, software-pipelined so PE
    # never sits behind the Act exp; heads 2/3 stats and the W_o projection
    # interleave into the PE slack of the Act-bound chain.
    def wo_steps(qt_lo, qt_hi):
        for qt in range(qt_lo, qt_hi):
            for ec in range(E // QC):
                ps = ps_sc.tile([P, QC], F32, tag="sc", name="ps_wo")
                for fc in range(FG // P):
                    yield lambda ps=ps, qt=qt, ec=ec, fc=fc: nc.tensor.matmul(
                        ps,
                        lhsT=att[:, fc, qt * P : (qt + 1) * P],
                        rhs=wos[:, fc, ec * QC : (ec + 1) * QC],
                        start=(fc == 0),
                        stop=(fc == FG // P - 1),
                        skip_group_check=True,
                    )
                ob = outp.tile([P, QC], F32, tag="ob")
                nc.vector.tensor_copy(ob[:], ps)
                nc.sync.dma_start(
                    out[qt * P : (qt + 1) * P, ec * QC : (ec + 1) * QC], ob[:]
                )

    LAG = 2
    head_order = [0, 1, 2, 3]
    # Fill schedule: head h's section drains head h+1's stats, one step per
    # kt slot (64 steps / 64 slots); the last head drains W_o instead.
    fills = {0: stats_its[1], 1: stats_its[2], 2: stats_its[3]}
    wo_pending = None
    for hi_idx, h in enumerate(head_order):
        for qc in range(NQC):
            qs = slice(qc * QC, (qc + 1) * QC)
            pv = ps_pv.tile([P, QC], F32, tag="pv")
            pts = {}

            def pv_mm(kt, pv=pv, h=h):
                nc.tensor.matmul(
                    pv[0 : Dh + 1, :],
                    lhsT=vau[:, kt, h, :],
                    rhs=pts.pop(kt)[:],
                    start=(kt == 0),
                    stop=(kt == ST - 1),
                    skip_group_check=True,
                )

            for kt in range(ST):
                ks = slice(kt * P, (kt + 1) * P)
                if hi_idx == HPC - 1:
                    drain(wo_pending, 1)
                else:
                    drain(fills[h], 1)
                ps = ps_sc.tile([P, QC], F32, tag="sc")
                nc.tensor.matmul(
                    ps, lhsT=kT[0 : Dh + 1, h, ks], rhs=qT[0 : Dh + 1, h, qs],
                    start=True, stop=True,
                )
                pt = ptp.tile([P, QC], F16, tag="pt")
                pts[kt] = pt
                nc.scalar.activation(pt[:], ps[:], AF.Exp, scale=SCALE)
                if kt >= LAG:
                    pv_mm(kt - LAG)
            for kt in range(ST - LAG, ST):
                pv_mm(kt)
            li = stage.tile([P, QC], F32R, tag="li")
            with nc.allow_low_precision(reason="1/l in f32r (~2^-12) is ample"):
                nc.vector.reciprocal(li[Dh : Dh + 1, :], pv[Dh : Dh + 1, :])
            pb = ps_sc.tile([P, QC], F32, tag="sc", name="pb")
            nc.tensor.matmul(
                pb[0:Dh, :], lhsT=ones_mat[Dh : Dh + 1, :], rhs=li[Dh : Dh + 1, :],
                start=True, stop=True,
            )
            bc = stage.tile([P, QC], F32, tag="bc")
            nc.scalar.copy(bc[0:Dh, :], pb[0:Dh, :])
            if h % 2 == 0:
                nc.vector.tensor_tensor(
                    att[0:Dh, h // 2, qs], pv[0:Dh, :], bc[0:Dh, :], ALU.mult
                )
            else:
                stg = stage.tile([P, QC], F32R, tag="att_stg")
                nc.vector.tensor_tensor(stg[0:Dh, :], pv[0:Dh, :], bc[0:Dh, :], ALU.mult)
                nc.sync.dma_start(att[Dh : 2 * Dh, h // 2, qs], stg[0:Dh, :])
            if hi_idx == HPC - 1:
                drain(wo_pending)
                wo_pending = wo_steps(4 * qc, 4 * qc + 4)
        if hi_idx == HPC - 1:
            drain(wo_pending)
        if hi_idx + 1 < HPC:
            nxt = head_order[hi_idx + 1]
            drain(stats_its[nxt])
            aug_prep(nxt)

    if debug:
        vau_d = nc.dram_tensor("vau_d", [P, ST, HPC, Dh + 1], F16, kind="ExternalOutput").ap()
        nc.sync.dma_start(vau_d, vau[:])
        att_d = nc.dram_tensor("att_d", [P, FG // P, S], F32, kind="ExternalOutput").ap()
        qT_d = nc.dram_tensor("qT_d", [P, HPC, S], F32, kind="ExternalOutput").ap()
        kT_d = nc.dram_tensor("kT_d", [P, HPC, S], F32, kind="ExternalOutput").ap()
        nc.sync.dma_start(att_d, att[:].bitcast(F32))
        nc.sync.dma_start(qT_d, qT[:].bitcast(F32))
        nc.sync.dma_start(kT_d, kT[:].bitcast(F32))
    ctx.close()


_NC = None


def _build(debug=False):
    global _NC
    if debug:
        nc = bacc.Bacc(
            "TRN2", target_bir_lowering=False, debug=False, num_devices=NCORES
        )
        with tile.TileContext(nc) as tc:
            _emit(tc, debug=True)
        nc.compile()
        return nc
    if _NC is None:
        nc = bacc.Bacc(
            "TRN2", target_bir_lowering=False, debug=False, num_devices=NCORES
        )
        with tile.TileContext(nc) as tc:
            _emit(tc)
        nc.compile()
        _NC = nc
    return _NC


def _prep_inputs(x, W_q, W_k, W_v, W_o):
    x = np.asarray(x, dtype=np.float32)
    W_q = np.asarray(W_q, dtype=np.float32)
    W_k = np.asarray(W_k, dtype=np.float32)
    W_v = np.asarray(W_v, dtype=np.float32)
    W_o = np.asarray(W_o, dtype=np.float32)

    xts = [np.ascontiguousarray(x[b].T) for b in range(B)]
    in_maps = []
    for c in range(NCORES):
        b, g = divmod(c, GROUPS)
        fg = slice(g * FG, (g + 1) * FG)
        in_maps.append(
            {
                "xt": xts[b],
                "wq": np.ascontiguousarray(W_q[:, fg]),
                "wk": np.ascontiguousarray(W_k[:, fg]),
                "wv": np.ascontiguousarray(W_v[:, fg]),
                "wo": np.ascontiguousarray(W_o[fg, :]),
            }
        )
    return in_maps


def run(inputs, **spmd_kwargs):
    nc = _build()
    in_maps = _prep_inputs(
        inputs["x"], inputs["W_q"], inputs["W_k"], inputs["W_v"], inputs["W_o"]
    )
    res = bass_utils.run_bass_kernel_spmd(
        nc, in_maps, core_ids=list(range(NCORES)), **spmd_kwargs
    )
    out = np.zeros((B, S, E), dtype=np.float32)
    for c in range(NCORES):
        out[c // GROUPS] += res.results[c]["out"]
    return out, res


def kernel(**inputs):
    out, _ = run(inputs)
    return out
